# revision 4
# baseline (speedup 1.0000x reference)
"""Multi-head attention (Vaswani) on Trainium2, head-parallel across 8 NeuronCores.

Problem shapes (hardcoded):
  h:   [B=2, G=2048, D=128] f32
  W_Q/W_K/W_V: [H=8, D=128, K=16] f32
  out: [B=2, H=8, G=2048, V=16] f32  = softmax(0.25 * (h@Wq) @ (h@Wk)^T) @ (h@Wv)

Sharding: one head per core (8 heads / 8 cores). Each core receives the full h
plus its head's weight slices, computes [B, G, V]; host stacks on the head axis.

Per-core plan (v2 — dual-engine exp, bf16 streams):
  All in transposed "compatT" orientation so the attention @ V contraction
  lands on the partition axis with no transposes of the big G x G matrix:
  1. hT[d, g] bf16 via DMA xbar transposes of bf16-converted h tiles.
  2. qT[16, g], kT[16, g] = Wq^T @ hT, Wk^T @ hT (bf16, K=16 contraction).
  3. v'[m, 17] chunks = (h_chunk @ Wv | ones column); the ones column makes the
     softmax denominator accumulate in output row 16 for free.
  4. Per key chunk m (128 keys) and q-slice (1024 wide): compatT[m, q] =
     k_m . q into psum; attnT = exp(0.25 * compatT) computed EITHER on the
     Scalar engine (exact spline exp) or, for `n_dve` of the 16 key chunks,
     on the Vector engine via a custom 8-stage DVE op
     (EXP_QUAD16_ANT: ((A x + B) x + C)^16 ~ exp(0.25 x), see _quad16_op) —
     splitting the exp roofline across two engines. oT[17, q] += v'^T @ attnT
     accumulates in psum over the 16 key chunks. Input staging for the next
     batch is interleaved into this loop.
  5. Transpose oT back in [17,128] blocks, scale rows by the reciprocal of
     the denominator row, one DMA per q-slice out.

The matmul streams run in bf16 (measured ~2 cols/cycle on HW); projections
also bf16 (fp32 matmuls are ~10x slower on the PE and dominate otherwise).
Measured end-to-end rel err ~8e-3 at n_dve=6 (gate is 2e-2).
"""

import numpy as np

B, G, D = 2, 2048, 128
H, K, V = 8, 16, 16
N_CORES = 8
P = 128
GT = G // P          # 16 key/query chunks of 128
QB = 512             # one fp32 PSUM bank of free dim
NQB = G // QB        # 4
VP1 = V + 1          # v' width (ones column appended)

DEFAULT_CFG = {
    "chunk_w": 1024,   # compat psum tile width (q-slice width)
    "pc_bufs": 2,      # compat psum buffers
    "at_bufs": 6,      # attnT sbuf buffers
    "n_dve": 5,        # key chunks per slice whose exp runs on DVE (of 16)
    "pack_pf": True,   # pack the 8 normalize transposes into one psum bank
    "reps": 1,         # repeat whole kernel body (for HW slope timing)
}

_CACHE = {}

# QUAD16 custom-DVE exp approximation: out = ((C0*x + C1)*x + C2)^16
# approximates exp(0.25*x) for raw compat logits x (the 0.25 scale is folded
# into the coefficients; the c0 freedom absorbs the global normalization,
# which cancels in softmax). 8 ALU stages: 4 (quadratic Horner) + 4 squarings.
# Coefficients minimax-optimized on the scaled-logit range [-3.5, 8.6]
# (values below only need |out| small, not accurate).
_QUAD16_RAW = (0.00227011 / 16.0, 0.06301235 / 4.0, 0.99802474)
_QUAD16 = None


def _quad16_op():
    """Register (once) and return the QUAD16 DveOp + call-site constants."""
    global _QUAD16
    if _QUAD16 is not None:
        return _QUAD16
    from concourse import dve_ops
    from concourse.dve_spec import Spec, Src0, C0, C1, C2, sq, lower
    from concourse.dve_uop import DveOpSpec

    A, Bc, Cc = _QUAD16_RAW

    def _ref(in0, in1, s0, s1, imm2):
        r = (in0.astype(np.float32) * s0 + s1) * in0 + imm2
        for _ in range(4):
            r = r * r
        return r

    body = sq(sq(sq(sq((Src0 * C0 + C1) * Src0 + C2))))
    spec = Spec(body=body, reference=_ref)
    name = "EXP_QUAD16_ANT"
    if not any(op.name == name for op in dve_ops.OPS):
        row = dve_ops._CUSTOM_DVE_ROW_BASE + len(dve_ops.OPS)
        assert row < 0x20
        shas = {}
        for ver in ("v3", "v4"):
            shas[ver] = DveOpSpec(
                name=name, opcode=row, uops=lower(spec, ver=ver),
                rd1_en=False).sha(ver)
        op = dve_ops.DveOp(name, spec, subdim=False, uops_sha=shas)
        dve_ops.OPS.append(op)
        dve_ops.CUSTOM_DVE_SPECS[name] = spec
        dve_ops._SUB_OPCODE_FOR_NAME[name] = row
    op = next(op for op in dve_ops.OPS if op.name == name)
    _QUAD16 = (op, {"s0": float(A), "s1": float(Bc), "imm2": float(Cc)})
    return _QUAD16


def _dve_chunks(n_dve):
    """Spread n_dve of the 16 key chunks evenly, avoiding chunk 0 (let the
    Scalar engine start immediately while DVE drains staging work)."""
    if n_dve <= 0:
        return set()
    return {1 + (i * 15) // n_dve for i in range(n_dve)}


def _build(cfg_key):
    cfg = dict(DEFAULT_CFG)
    cfg.update(dict(cfg_key))
    import concourse.bacc as bacc
    import concourse.mybir as mybir
    from concourse.tile import TileContext
    from concourse.masks import make_identity

    f32 = mybir.dt.float32
    bf16 = mybir.dt.bfloat16
    EXP = mybir.ActivationFunctionType.Exp
    QUAD16, qc = _quad16_op()

    nc = bacc.Bacc("TRN2", debug=False, enable_asserts=False,
                   target_bir_lowering=False)
    h_d = nc.dram_tensor("h", [B, G, D], f32, kind="ExternalInput").ap()
    wq_d = nc.dram_tensor("wq", [D, K], f32, kind="ExternalInput").ap()
    wk_d = nc.dram_tensor("wk", [D, K], f32, kind="ExternalInput").ap()
    wv_d = nc.dram_tensor("wv", [D, V], f32, kind="ExternalInput").ap()
    out_d = nc.dram_tensor("out", [B, G, V], f32, kind="ExternalOutput").ap()

    CW = cfg["chunk_w"]
    NCW = G // CW        # q-slices per batch
    dve_set = _dve_chunks(cfg["n_dve"])

    with TileContext(nc) as tc:
        with tc.tile_pool(name="const", bufs=1) as cpool, \
             tc.tile_pool(name="sc", bufs=2, space="PSUM") as scpool, \
             tc.tile_pool(name="pc", bufs=cfg["pc_bufs"],
                          space="PSUM") as pcpool, \
             tc.tile_pool(name="po", bufs=1, space="PSUM") as popool, \
             tc.tile_pool(name="att", bufs=cfg["at_bufs"]) as apool:
            ident = cpool.tile([P, P], f32)
            make_identity(nc, ident)
            warm = cpool.tile([P, 1], f32)
            nc.scalar.activation(warm, ident[:, 0:1], EXP)
            w_sb = cpool.tile([D, 3 * K], f32)
            w_r = cpool.tile([D, 3 * K], bf16)

            def load_w():
                nc.sync.dma_start(w_sb[:, 0:K], wq_d)
                nc.sync.dma_start(w_sb[:, K:2 * K], wk_d)
                nc.sync.dma_start(w_sb[:, 2 * K:3 * K], wv_d)
                nc.vector.tensor_copy(w_r, w_sb)

            hA_b, hAb_b, hT_b, qkT_b, kTp_b, vp_b, ob_b = ([], [], [], [],
                                                           [], [], [])
            for b in range(B):
                hA_b.append(cpool.tile([P, G], f32, name=f"hA{b}"))
                hAb_b.append(cpool.tile([P, G], bf16, name=f"hAb{b}"))
                hT_b.append(cpool.tile([P, G], bf16, name=f"hT{b}"))
                qkT_b.append(cpool.tile([K, G], bf16, name=f"qkT{b}"))
                kTp_b.append(cpool.tile([K, G], bf16, name=f"kTp{b}"))
                vp_b.append(cpool.tile([P, GT * VP1], bf16, name=f"vp{b}"))
                ob_b.append(cpool.tile([P, GT * V], f32, name=f"ob{b}"))

            def init_vp():
                # ones columns are static; vproj only overwrites cols 0..15
                for b in range(B):
                    for t in range(GT):
                        nc.vector.memset(
                            vp_b[b][:, t * VP1 + V:(t + 1) * VP1], 1.0)

            def phase1_ops(b):
                """Closure list for batch b's input staging, in dependency
                order; popped a few at a time inside the previous batch's
                main loop so the work fills engine gaps."""
                hA, hAb, hT = hA_b[b], hAb_b[b], hT_b[b]
                qkT, kTp, vp = qkT_b[b], kTp_b[b], vp_b[b]

                def dmaq(qq):
                    nc.sync.dma_start(
                        hA[:, qq * 4 * P:(qq + 1) * 4 * P].rearrange(
                            "p (t d) -> p t d", t=4),
                        h_d[b, qq * 4 * P:(qq + 1) * 4 * P, :].rearrange(
                            "(t p) d -> p t d", p=P))

                def conv(qq):
                    nc.vector.tensor_copy(
                        hAb[:, qq * 4 * P:(qq + 1) * 4 * P],
                        hA[:, qq * 4 * P:(qq + 1) * 4 * P])

                def dmaT(t):
                    nc.sync.dma_start_transpose(
                        hT[:, t * P:(t + 1) * P], hAb[:, t * P:(t + 1) * P])

                def proj(qb, w0, dst):
                    sl = slice(qb * QB, (qb + 1) * QB)
                    pq = scpool.tile([P, QB], f32, tag="s", name="pq")
                    nc.tensor.matmul(pq[0:K, :], w_r[:, w0:w0 + K],
                                     hT[:, sl], start=True, stop=True)
                    nc.vector.tensor_copy(dst[0:K, sl], pq[0:K, :])

                def vproj4(qq):
                    # 4 key chunks' v-projections into one psum tile, one copy
                    pvv = scpool.tile([P, QB], f32, tag="s", name="pvv")
                    for j in range(4):
                        t = 4 * qq + j
                        nc.tensor.matmul(pvv[:, j * V:(j + 1) * V],
                                         hT[:, t * P:(t + 1) * P],
                                         w_r[:, 2 * K:3 * K],
                                         start=True, stop=True)
                    nc.vector.tensor_copy(
                        vp[:, 4 * qq * VP1:(4 * qq + 4) * VP1].rearrange(
                            "p (t w) -> p t w", t=4)[:, :, 0:V],
                        pvv[:, 0:4 * V].rearrange("p (t v) -> p t v", t=4))

                ops = [lambda: dmaq(0), lambda: dmaq(1),
                       lambda: dmaq(2), lambda: dmaq(3)]
                for qq in range(NQB):
                    ops.append(lambda qq=qq: conv(qq))
                    for t in range(4 * qq, 4 * qq + 4):
                        ops.append(lambda t=t: dmaT(t))
                    ops.append(lambda qq=qq: proj(qq, 0, qkT))
                    ops.append(lambda qq=qq: proj(qq, K, kTp))
                    ops.append(lambda qq=qq: vproj4(qq))
                return ops

            units = [(rr, bb) for rr in range(cfg["reps"])
                     for bb in range(B)]
            first = phase1_ops(units[0][1])
            first = (first[0:2] + [load_w] + first[2:4] + [init_vp]
                     + first[4:])
            # prefix: everything through qq=1's projections (q-slice 0 needs
            # qkT[0:1024]; later kTp/vp chunks are ordered via the 3-per-chunk
            # interleave below, which stays ahead of the chunk loop)
            npre = 22
            for op in first[:npre]:
                op()
            pending = first[npre:]
            for ui, (rep, b) in enumerate(units):
                qkT, kTp, vp, ob_all = (qkT_b[b], kTp_b[b], vp_b[b],
                                        ob_b[b])
                if ui + 1 < len(units):
                    pending = pending + phase1_ops(units[ui + 1][1])

                for si in range(NCW):
                    q0 = si * CW
                    width = CW
                    oT = popool.tile([VP1, CW], f32, tag="oT",
                                     name="oT")[:, 0:width]
                    for t in range(GT):
                        v_sl = vp[:, t * VP1:(t + 1) * VP1]
                        cps = pcpool.tile([P, CW], f32, tag="c",
                                          name="cps")[:, 0:width]
                        kT_sl = kTp[0:K, t * P:(t + 1) * P]
                        for j in range(width // QB):
                            nc.tensor.matmul(
                                cps[:, j * QB:(j + 1) * QB], kT_sl,
                                qkT[0:K, q0 + j * QB:q0 + (j + 1) * QB],
                                start=True, stop=True)
                        at = apool.tile([P, CW], bf16, tag="at",
                                        name="at")[:, 0:width]
                        if t in dve_set:
                            nc.vector._custom_dve(
                                QUAD16, out=at, in0=cps,
                                s0=qc["s0"], s1=qc["s1"], imm2=qc["imm2"])
                        else:
                            nc.scalar.activation(at, cps, EXP, scale=0.25)
                        for j in range(width // QB):
                            nc.tensor.matmul(
                                oT[:, j * QB:(j + 1) * QB], v_sl,
                                at[:, j * QB:(j + 1) * QB],
                                start=(t == 0), stop=(t == GT - 1))
                        # emit a few staged ops for the NEXT batch; end-of-
                        # chunk placement keeps them behind this chunk's
                        # matmuls in the PE queue while still preceding
                        # every consumer
                        for _ in range(3):
                            if pending:
                                pending.pop(0)()

                    # normalize this q-slice
                    oT_sb = apool.tile([VP1, CW], f32, tag="oTsb",
                                       name="oT_sb")[:, 0:width]
                    half = width // 2
                    nc.vector.tensor_copy(oT_sb[:, 0:half], oT[:, 0:half])
                    nc.vector.tensor_copy(oT_sb[:, half:width],
                                          oT[:, half:width])
                    for tl in range(width // P):
                        tg = (q0 + tl * P) // P
                        pf = scpool.tile([P, QB], f32, tag="s", name="pf")
                        nc.tensor.transpose(
                            pf[:, 0:VP1],
                            oT_sb[:, tl * P:(tl + 1) * P],
                            ident[:VP1, :VP1])
                        rcp = apool.tile([P, 1], f32, tag="rcp",
                                         name="rcp")
                        nc.vector.reciprocal(rcp, pf[:, V:V + 1])
                        nc.vector.tensor_scalar_mul(
                            ob_all[:, tg * V:(tg + 1) * V],
                            pf[:, 0:V], rcp)

                    # per-slice out DMA so the store overlaps the next
                    nc.sync.dma_start(
                        out_d[b, q0:q0 + width, :].rearrange(
                            "(t p) v -> p t v", p=P),
                        ob_all[:, (q0 // P) * V:((q0 + width) // P) * V]
                        .rearrange("p (t v) -> p t v", t=width // P))

                for op in pending:
                    op()
                pending = []

    nc.compile()
    return nc


def _get(cfg=None):
    cfg = cfg or {}
    key = tuple(sorted({**DEFAULT_CFG, **cfg}.items()))
    if key not in _CACHE:
        _CACHE[key] = _build(key)
    return _CACHE[key]


def _in_maps(h, W_Q, W_K, W_V):
    h = np.ascontiguousarray(np.asarray(h, dtype=np.float32))
    W_Q = np.asarray(W_Q, dtype=np.float32)
    W_K = np.asarray(W_K, dtype=np.float32)
    W_V = np.asarray(W_V, dtype=np.float32)
    return [
        {"h": h, "wq": np.ascontiguousarray(W_Q[c]),
         "wk": np.ascontiguousarray(W_K[c]),
         "wv": np.ascontiguousarray(W_V[c])}
        for c in range(N_CORES)
    ]


def kernel(h, W_Q, W_K, W_V, cfg=None, **run_kwargs):
    from concourse import bass_utils
    nc = _get(cfg)
    res = bass_utils.run_bass_kernel_spmd(
        nc, _in_maps(h, W_Q, W_K, W_V),
        core_ids=list(range(N_CORES)), **run_kwargs)
    out = np.stack([res.results[c]["out"] for c in range(N_CORES)], axis=1)
    kernel.last_results = res
    return out


# revision 17
# speedup vs baseline: 1.0309x; 1.0309x over previous
"""Multi-head attention (Vaswani) on Trainium2, head-parallel across 8 NeuronCores.

Problem shapes (hardcoded):
  h:   [B=2, G=2048, D=128] f32
  W_Q/W_K/W_V: [H=8, D=128, K=16] f32
  out: [B=2, H=8, G=2048, V=16] f32  = softmax(0.25 * (h@Wq) @ (h@Wk)^T) @ (h@Wv)

Sharding: one head per core (8 heads / 8 cores). Each core receives the full h
plus its head's weight slices, computes [B, G, V]; host stacks on the head axis.

Per-core plan (v2 — dual-engine exp, bf16 streams):
  All in transposed "compatT" orientation so the attention @ V contraction
  lands on the partition axis with no transposes of the big G x G matrix:
  1. hT[d, g] bf16 via DMA xbar transposes of bf16-converted h tiles.
  2. qT[16, g], kT[16, g] = Wq^T @ hT, Wk^T @ hT (bf16, K=16 contraction).
  3. v'[m, 17] chunks = (h_chunk @ Wv | ones column); the ones column makes the
     softmax denominator accumulate in output row 16 for free.
  4. Per key chunk m (128 keys) and q-slice (1024 wide): compatT[m, q] =
     k_m . q into psum; attnT = exp(0.25 * compatT) computed EITHER on the
     Scalar engine (exact spline exp) or, for `n_dve` of the 16 key chunks,
     on the Vector engine via a custom 8-stage DVE op
     (EXP_QUAD16_ANT: ((A x + B) x + C)^16 ~ exp(0.25 x), see _quad16_op) —
     splitting the exp roofline across two engines. oT[17, q] += v'^T @ attnT
     accumulates in psum over the 16 key chunks. Input staging for the next
     batch is interleaved into this loop.
  5. Transpose oT back in [17,128] blocks, scale rows by the reciprocal of
     the denominator row, one DMA per q-slice out.

The matmul streams run in bf16 (measured ~2 cols/cycle on HW); projections
also bf16 (fp32 matmuls are ~10x slower on the PE and dominate otherwise).
Measured end-to-end rel err ~8e-3 at n_dve=6 (gate is 2e-2).
"""

import numpy as np

B, G, D = 2, 2048, 128
H, K, V = 8, 16, 16
N_CORES = 8
P = 128
GT = G // P          # 16 key/query chunks of 128
QB = 512             # one fp32 PSUM bank of free dim
NQB = G // QB        # 4
VP1 = V + 1          # v' width (ones column appended)

DEFAULT_CFG = {
    "chunk_w": 1024,   # compat psum tile width (q-slice width)
    "pc_bufs": 3,      # compat psum buffers (lookahead-2 pipeline)
    "at_bufs": 6,      # attnT sbuf buffers
    "n_dve": 5,        # key chunks per slice whose exp runs on DVE (of 16)
    "pack_pf": True,   # pack the 8 normalize transposes into one psum bank
    "reps": 1,         # repeat whole kernel body (for HW slope timing)
}

_CACHE = {}

# QUAD16 custom-DVE exp approximation: out = ((C0*x + C1)*x + C2)^16
# approximates exp(0.25*x) for raw compat logits x (the 0.25 scale is folded
# into the coefficients; the c0 freedom absorbs the global normalization,
# which cancels in softmax). 8 ALU stages: 4 (quadratic Horner) + 4 squarings.
# Coefficients optimized directly against the end-to-end output L2 (smooth
# systematic exp error largely cancels through the softmax normalization, so
# this beats minimax-on-exp by ~6x; validated on all heads / assignments).
_QUAD16_RAW = (0.00224192 / 16.0, 0.06216735 / 4.0, 0.99982926)
_QUAD16 = None


def _quad16_op():
    """Register (once) and return the QUAD16 DveOp + call-site constants."""
    global _QUAD16
    if _QUAD16 is not None:
        return _QUAD16
    from concourse import dve_ops
    from concourse.dve_spec import Spec, Src0, C0, C1, C2, sq, lower
    from concourse.dve_uop import DveOpSpec

    A, Bc, Cc = _QUAD16_RAW

    def _ref(in0, in1, s0, s1, imm2):
        r = (in0.astype(np.float32) * s0 + s1) * in0 + imm2
        for _ in range(4):
            r = r * r
        return r

    body = sq(sq(sq(sq((Src0 * C0 + C1) * Src0 + C2))))
    spec = Spec(body=body, reference=_ref)
    name = "EXP_QUAD16_ANT"
    if not any(op.name == name for op in dve_ops.OPS):
        row = dve_ops._CUSTOM_DVE_ROW_BASE + len(dve_ops.OPS)
        assert row < 0x20
        shas = {}
        for ver in ("v3", "v4"):
            shas[ver] = DveOpSpec(
                name=name, opcode=row, uops=lower(spec, ver=ver),
                rd1_en=False).sha(ver)
        op = dve_ops.DveOp(name, spec, subdim=False, uops_sha=shas)
        dve_ops.OPS.append(op)
        dve_ops.CUSTOM_DVE_SPECS[name] = spec
        dve_ops._SUB_OPCODE_FOR_NAME[name] = row
    op = next(op for op in dve_ops.OPS if op.name == name)
    _QUAD16 = (op, {"s0": float(A), "s1": float(Bc), "imm2": float(Cc)})
    return _QUAD16


def _dve_chunks(n_dve):
    """Spread n_dve of the 16 key chunks evenly, avoiding chunk 0 (let the
    Scalar engine start immediately while DVE drains staging work)."""
    if n_dve <= 0:
        return set()
    return {1 + (i * 15) // n_dve for i in range(n_dve)}


def _build(cfg_key):
    cfg = dict(DEFAULT_CFG)
    cfg.update(dict(cfg_key))
    import concourse.bacc as bacc
    import concourse.mybir as mybir
    from concourse.tile import TileContext
    from concourse.masks import make_identity

    f32 = mybir.dt.float32
    bf16 = mybir.dt.bfloat16
    EXP = mybir.ActivationFunctionType.Exp
    QUAD16, qc = _quad16_op()

    nc = bacc.Bacc("TRN2", debug=False, enable_asserts=False,
                   target_bir_lowering=False)
    h_d = nc.dram_tensor("h", [B, G, D], f32, kind="ExternalInput").ap()
    wq_d = nc.dram_tensor("wq", [D, K], f32, kind="ExternalInput").ap()
    wk_d = nc.dram_tensor("wk", [D, K], f32, kind="ExternalInput").ap()
    wv_d = nc.dram_tensor("wv", [D, V], f32, kind="ExternalInput").ap()
    out_d = nc.dram_tensor("out", [B, G, V], f32, kind="ExternalOutput").ap()

    CW = cfg["chunk_w"]
    NCW = G // CW        # q-slices per batch
    dve_set = _dve_chunks(cfg["n_dve"])

    with TileContext(nc) as tc:
        with tc.tile_pool(name="const", bufs=1) as cpool, \
             tc.tile_pool(name="pc", bufs=cfg["pc_bufs"],
                          space="PSUM") as pcpool, \
             tc.tile_pool(name="po", bufs=1, space="PSUM") as popool, \
             tc.tile_pool(name="att", bufs=cfg["at_bufs"]) as apool:
            ident = cpool.tile([P, P], f32)
            make_identity(nc, ident)
            warm = cpool.tile([P, 1], f32)
            nc.scalar.activation(warm, ident[:, 0:1], EXP)
            w_sb = cpool.tile([D, 3 * K], f32)
            w_r = cpool.tile([D, 3 * K], bf16)

            def load_w():
                nc.sync.dma_start(w_sb[:, 0:K], wq_d)
                nc.sync.dma_start(w_sb[:, K:2 * K], wk_d)
                nc.sync.dma_start(w_sb[:, 2 * K:3 * K], wv_d)
                nc.vector.tensor_copy(w_r, w_sb)

            hA_b, hAb_b, hT_b, qk_b, vp_b, ob_b = ([], [], [], [], [], [])
            for b in range(B):
                hA_b.append(cpool.tile([P, G], f32, name=f"hA{b}"))
                hAb_b.append(cpool.tile([P, G], bf16, name=f"hAb{b}"))
                hT_b.append(cpool.tile([P, G], bf16, name=f"hT{b}"))
                # rows 0:16 = qT, rows 16:32 = kT (one proj matmul + copy)
                qk_b.append(cpool.tile([2 * K, G], bf16, name=f"qk{b}"))
                vp_b.append(cpool.tile([P, GT * VP1], bf16, name=f"vp{b}"))
                ob_b.append(cpool.tile([P, GT * V], f32, name=f"ob{b}"))

            def init_vp():
                # ones columns are static; vproj only overwrites cols 0..15.
                # GPSIMD is otherwise idle, keep this off the Vector engine.
                for b in range(B):
                    for t in range(GT):
                        nc.gpsimd.memset(
                            vp_b[b][:, t * VP1 + V:(t + 1) * VP1], 1.0)

            def phase1_ops(b):
                """Closure list for batch b's input staging, in dependency
                order; popped a few at a time inside the previous batch's
                main loop so the work fills engine gaps."""
                hA, hAb, hT = hA_b[b], hAb_b[b], hT_b[b]
                qk, vp = qk_b[b], vp_b[b]

                def dmaq(qq):
                    nc.sync.dma_start(
                        hA[:, qq * 4 * P:(qq + 1) * 4 * P].rearrange(
                            "p (t d) -> p t d", t=4),
                        h_d[b, qq * 4 * P:(qq + 1) * 4 * P, :].rearrange(
                            "(t p) d -> p t d", p=P))

                def conv(qq):
                    nc.vector.tensor_copy(
                        hAb[:, qq * 4 * P:(qq + 1) * 4 * P],
                        hA[:, qq * 4 * P:(qq + 1) * 4 * P])

                def dmaT(qq):
                    # one xbar transpose per 512-col group: [g,(t d)] tiles
                    # land as hT[d, t, g] (out[p, s, x] = in[x, s*128+p])
                    nc.sync.dma_start_transpose(
                        hT[:, qq * 4 * P:(qq + 1) * 4 * P].rearrange(
                            "p (t g) -> p t g", t=4),
                        hAb[:, qq * 4 * P:(qq + 1) * 4 * P])

                def proj(qb):
                    # q and k projections in one matmul (adjacent weight
                    # columns) and one psum->sbuf copy
                    sl = slice(qb * QB, (qb + 1) * QB)
                    pq = pcpool.tile([P, CW], f32, tag="c", name="pq")
                    nc.tensor.matmul(pq[0:2 * K, 0:QB], w_r[:, 0:2 * K],
                                     hT[:, sl], start=True, stop=True)
                    nc.vector.tensor_copy(qk[0:2 * K, sl],
                                          pq[0:2 * K, 0:QB])

                def vproj4(qq):
                    # 4 key chunks' v-projections into one psum tile, one copy
                    pvv = pcpool.tile([P, CW], f32, tag="c", name="pvv")
                    for j in range(4):
                        t = 4 * qq + j
                        nc.tensor.matmul(pvv[:, j * V:(j + 1) * V],
                                         hT[:, t * P:(t + 1) * P],
                                         w_r[:, 2 * K:3 * K],
                                         start=True, stop=True)
                    nc.vector.tensor_copy(
                        vp[:, 4 * qq * VP1:(4 * qq + 4) * VP1].rearrange(
                            "p (t w) -> p t w", t=4)[:, :, 0:V],
                        pvv[:, 0:4 * V].rearrange("p (t v) -> p t v", t=4))

                # interleave each dma with its dependents so the startup
                # critical path isn't gated by all four h loads up front
                ops = []
                for qq in range(NQB):
                    ops.append(lambda qq=qq: dmaq(qq))
                    ops.append(lambda qq=qq: conv(qq))
                    ops.append(lambda qq=qq: dmaT(qq))
                    if qq >= 1:
                        ops.append(lambda qq=qq: proj(qq))
                        ops.append(lambda qq=qq: vproj4(qq))
                    if qq == 1:
                        ops.insert(-2, lambda: proj(0))
                        ops.append(lambda: vproj4(0))
                return ops

            units = [(rr, bb) for rr in range(cfg["reps"])
                     for bb in range(B)]
            first = phase1_ops(units[0][1])
            first = (first[0:1] + [load_w] + first[1:3] + [init_vp]
                     + first[3:])
            # prefix: everything through qq<=1 staging + proj(0), proj(1),
            # vproj4(1), vproj4(0) (q-slice 0 chunk reads need qk[:,0:1024]
            # and vp chunks 0-7; the interleave stays ahead for the rest)
            npre = 13
            for op in first[:npre]:
                op()
            pending = first[npre:]

            def _normalize(ui, b, q0, oT):
                ob_all = ob_b[b]
                width = CW
                oT_sb = apool.tile([VP1, CW], f32, tag="oTsb",
                                   name="oT_sb")
                nc.vector.tensor_copy(oT_sb, oT)
                NT = width // P
                if cfg["pack_pf"]:
                    # all NT transposes into one psum bank; one strided
                    # reciprocal covers every denominator column
                    pf = pcpool.tile([P, CW], f32, tag="c", name="pf")
                    for tl in range(NT):
                        nc.tensor.transpose(
                            pf[:, tl * VP1:(tl + 1) * VP1],
                            oT_sb[:, tl * P:(tl + 1) * P],
                            ident[:VP1, :VP1])
                    rcp = apool.tile([P, NT], f32, tag="rcp", name="rcp")
                    nc.vector.reciprocal(rcp, pf[:, V:NT * VP1:VP1])
                    for tl in range(NT):
                        tg = (q0 + tl * P) // P
                        nc.vector.tensor_scalar_mul(
                            ob_all[:, tg * V:(tg + 1) * V],
                            pf[:, tl * VP1:tl * VP1 + V],
                            rcp[:, tl:tl + 1])
                else:
                    for tl in range(NT):
                        tg = (q0 + tl * P) // P
                        pf = pcpool.tile([P, CW], f32, tag="c", name="pf")
                        nc.tensor.transpose(
                            pf[:, 0:VP1],
                            oT_sb[:, tl * P:(tl + 1) * P],
                            ident[:VP1, :VP1])
                        rcp = apool.tile([P, 1], f32, tag="rcp",
                                         name="rcp")
                        nc.vector.reciprocal(rcp, pf[:, V:V + 1])
                        nc.vector.tensor_scalar_mul(
                            ob_all[:, tg * V:(tg + 1) * V],
                            pf[:, 0:V], rcp)

                # per-slice out DMA so the store overlaps the next
                nc.sync.dma_start(
                    out_d[b, q0:q0 + width, :].rearrange(
                        "(t p) v -> p t v", p=P),
                    ob_all[:, (q0 // P) * V:((q0 + width) // P) * V]
                    .rearrange("p (t v) -> p t v", t=width // P))

            # flattened chunk schedule: (unit, batch, slice q0, key chunk).
            # compat for chunk ci+1 is emitted BEFORE av(ci) so the PE queue
            # never head-of-line blocks the next exp behind an av that waits
            # on the current exp.
            sched = [(ui, b, si * CW, t)
                     for ui, (rep, b) in enumerate(units)
                     for si in range(NCW) for t in range(GT)]
            cps_tiles = {}

            def emit_compat(ci):
                ui, b, q0, t = sched[ci]
                qk = qk_b[b]
                cps = pcpool.tile([P, CW], f32, tag="c", name="cps")
                kT_sl = qk[K:2 * K, t * P:(t + 1) * P]
                for j in range(CW // QB):
                    nc.tensor.matmul(
                        cps[:, j * QB:(j + 1) * QB], kT_sl,
                        qk[0:K, q0 + j * QB:q0 + (j + 1) * QB],
                        start=True, stop=True)
                cps_tiles[ci] = cps

            oT = None
            width = CW
            for ci, (ui, b, q0, t) in enumerate(sched):
                vp, ob_all = vp_b[b], ob_b[b]
                if ci == 0:
                    emit_compat(0)
                    emit_compat(1)
                if t == 0 and q0 == 0 and ui + 1 < len(units):
                    pending = pending + phase1_ops(units[ui + 1][1])
                if ci + 2 < len(sched):
                    emit_compat(ci + 2)
                cps = cps_tiles.pop(ci)
                at = apool.tile([P, CW], bf16, tag="at", name="at")
                if t in dve_set:
                    nc.vector._custom_dve(
                        QUAD16, out=at, in0=cps,
                        s0=qc["s0"], s1=qc["s1"], imm2=qc["imm2"])
                else:
                    nc.scalar.activation(at, cps, EXP, scale=0.25)
                if t == 0:
                    oT = popool.tile([VP1, CW], f32, tag="oT", name="oT")
                v_sl = vp[:, t * VP1:(t + 1) * VP1]
                for j in range(width // QB):
                    nc.tensor.matmul(
                        oT[:, j * QB:(j + 1) * QB], v_sl,
                        at[:, j * QB:(j + 1) * QB],
                        start=(t == 0), stop=(t == GT - 1))
                # emit a few staged ops for the NEXT batch; end-of-chunk
                # placement keeps them behind this chunk's matmuls in the
                # PE queue while still preceding every consumer
                for _ in range(3):
                    if pending:
                        pending.pop(0)()
                if t == GT - 1:
                    _normalize(ui, b, q0, oT)

            for op in pending:
                op()

    nc.compile()
    return nc


def _get(cfg=None):
    cfg = cfg or {}
    key = tuple(sorted({**DEFAULT_CFG, **cfg}.items()))
    if key not in _CACHE:
        _CACHE[key] = _build(key)
    return _CACHE[key]


def _in_maps(h, W_Q, W_K, W_V):
    h = np.ascontiguousarray(np.asarray(h, dtype=np.float32))
    W_Q = np.asarray(W_Q, dtype=np.float32)
    W_K = np.asarray(W_K, dtype=np.float32)
    W_V = np.asarray(W_V, dtype=np.float32)
    return [
        {"h": h, "wq": np.ascontiguousarray(W_Q[c]),
         "wk": np.ascontiguousarray(W_K[c]),
         "wv": np.ascontiguousarray(W_V[c])}
        for c in range(N_CORES)
    ]


def kernel(h, W_Q, W_K, W_V, cfg=None, **run_kwargs):
    from concourse import bass_utils
    nc = _get(cfg)
    res = bass_utils.run_bass_kernel_spmd(
        nc, _in_maps(h, W_Q, W_K, W_V),
        core_ids=list(range(N_CORES)), **run_kwargs)
    out = np.stack([res.results[c]["out"] for c in range(N_CORES)], axis=1)
    kernel.last_results = res
    return out
